# revision 4
# baseline (speedup 1.0000x reference)
"""Trainium2 Bass kernel for nn_BaseEncoder (ragged entity-pair encoder).

Contract: kernel(**inputs) takes the FULL unsharded inputs (numpy) and
returns the FULL output [B, Q, E, E, R] float32.

Sharding: B*Q = 8 independent (batch, query) pairs -> one per NeuronCore.
Small weights (W_head / W_tail / prototypes-for-that-b) are replicated.

Host-side prep per core (cheap, index/layout only):
  - gather the E*M mention rows of the per-query attention and sum over the
    M=2 mentions (the /2 and /NH scalings cancel in the later row-softmax-
    style normalization, so they are dropped),
  - layout At[l, e*NH + h] (h innermost) in bf16 so the device pairwise
    product runs in the DVE 2x perf mode with stride-1 operands,
  - entity means ent = mean_m seq[pos] (transposed to entT),
  - prototypes for this b, reshaped/transposed to [2H, RP],
  - constant 0/1 selection matrices selH/selT that broadcast the entity
    projections over pair columns via a PE matmul (bias-add on PE).

Device kernel per core:
  P[l, e, f, h] = At[l,e,h] * At[l,f,h]                       (VectorE, bf16 2x)
  mul[l, ef]    = tree-sum_h P                                (VectorE + GpSimd)
  S[ef]   = sum_{l,h} At*At   (Gram)                          (TensorE)
  ctxT[h', ef] = sum_l seq[l, h'] * mul[l, ef]                (TensorE)
  ctxnT = ctxT * (1/S)                                        (VectorE/GpSimd)
  epHT[e', h''] = sum_h' entT[h', e'] W[h', h'']              (TensorE)
  proj psum = selH/T-bias MM + sum_kt W2^T ctxnT              (TensorE)
  cand = tanh(psum)                                           (ScalarE)
  scores[ef, rp] = sum_d candT[d, ef] * protoT[d, rp]         (TensorE)
  out[ef, r] = max_p scores[ef, r*10+p]                       (VectorE)
"""

import numpy as np

B, Q, L, H, E, M, R, P, NH = 2, 4, 1024, 768, 32, 2, 5, 10, 12
NCORES = 8
LT = L // 128          # 8 l-tiles
HT = H // 128          # 6 tiles of 128 along a hidden dim
EF = E * E             # 1024 entity pairs
RP = R * P             # 50 prototype rows
HC = EF // 2           # 512-wide ef chunk (= one PSUM bank of fp32)
EC = E // 2            # 16 e-values per chunk

_CACHE = {}


def _build_program():
    import concourse.mybir as mybir
    import concourse.tile as tile
    from concourse import bacc

    f32 = mybir.dt.float32
    bf16 = mybir.dt.bfloat16
    nc = bacc.Bacc("TRN2", target_bir_lowering=False, debug=False,
                   num_devices=NCORES)

    at_d = nc.dram_tensor("at", [L, E * NH], bf16, kind="ExternalInput").ap()
    seq_d = nc.dram_tensor("seq", [L, H], bf16, kind="ExternalInput").ap()
    entT_d = nc.dram_tensor("entT", [H, E], bf16, kind="ExternalInput").ap()
    wh_d = nc.dram_tensor("wh", [2 * H, H], bf16, kind="ExternalInput").ap()
    wt_d = nc.dram_tensor("wt", [2 * H, H], bf16, kind="ExternalInput").ap()
    ptT_d = nc.dram_tensor("ptT", [2 * H, RP], bf16, kind="ExternalInput").ap()
    sel_d = nc.dram_tensor("sel", [E, 2 * EF], bf16, kind="ExternalInput").ap()
    out_d = nc.dram_tensor("out", [EF, R], f32, kind="ExternalOutput").ap()

    with tile.TileContext(nc) as tc:
        _emit(tc, mybir, at_d, seq_d, entT_d, wh_d, wt_d, ptT_d, sel_d, out_d)

    nc.compile()
    return nc


def _emit(tc, mybir, at_d, seq_d, entT_d, wh_d, wt_d, ptT_d, sel_d, out_d):
    nc = tc.nc
    f32 = mybir.dt.float32
    bf16 = mybir.dt.bfloat16

    Alu = mybir.AluOpType
    Act = mybir.ActivationFunctionType
    Ax = mybir.AxisListType
    from concourse.masks import make_identity

    import contextlib
    ctx = contextlib.ExitStack()
    with ctx:
        const = ctx.enter_context(tc.tile_pool(name="const", bufs=1))
        big = ctx.enter_context(tc.tile_pool(name="big", bufs=1))
        mulp = ctx.enter_context(tc.tile_pool(name="mulp", bufs=12))
        prodp = ctx.enter_context(tc.tile_pool(name="prodp", bufs=2))
        treep = ctx.enter_context(tc.tile_pool(name="treep", bufs=2))
        candp = ctx.enter_context(tc.tile_pool(name="candp", bufs=14))
        ctxp = ctx.enter_context(tc.tile_pool(name="ctxp", bufs=2))
        # PSUM: 8 banks statically split into tags
        #   "ctx": 6 x 1 bank   (per-chunk ctx accumulators; later proj-B)
        #   "sg":  1 x 1 bank   (S-gram, recS broadcast, even proj-A, tp)
        #   "tail": 1 x 1 bank  (ep, odd proj-A groups, scores)
        psum = ctx.enter_context(tc.tile_pool(name="psum", bufs=1, space="PSUM"))

        # ---------------- input loads ----------------
        at_sb = big.tile([128, LT, E * NH], bf16, tag="at_sb")
        at_r = at_d.rearrange("(t p) n -> p t n", p=128)
        for lt in range(LT):
            nc.sync.dma_start(out=at_sb[:, lt, :], in_=at_r[:, lt, :])
        seq_sb = big.tile([128, LT, H], bf16, tag="seq_sb")
        nc.sync.dma_start(out=seq_sb, in_=seq_d.rearrange("(t p) n -> p t n", p=128))
        entT_sb = const.tile([128, HT, E], bf16, tag="entT_sb")
        nc.sync.dma_start(out=entT_sb, in_=entT_d.rearrange("(t p) n -> p t n", p=128))
        ptT_sb = const.tile([128, 2 * HT, RP], bf16, tag="ptT_sb")
        nc.sync.dma_start(out=ptT_sb, in_=ptT_d.rearrange("(t p) n -> p t n", p=128))
        wh_sb = big.tile([128, 2 * HT, H], bf16, tag="wh_sb")
        nc.sync.dma_start(out=wh_sb, in_=wh_d.rearrange("(t p) n -> p t n", p=128))
        wt_sb = big.tile([128, 2 * HT, H], bf16, tag="wt_sb")
        nc.sync.dma_start(out=wt_sb, in_=wt_d.rearrange("(t p) n -> p t n", p=128))
        sel_sb = const.tile([E, 2 * EF], bf16, tag="sel_sb")
        nc.sync.dma_start(out=sel_sb, in_=sel_d)

        ones_row = const.tile([1, 128], f32, tag="ones_row")
        nc.vector.memset(ones_row, 1.0)
        ident = const.tile([RP, RP], f32, tag="ident")
        make_identity(nc, ident)
        recd = nc.dram_tensor("recd", [E, E], f32).ap()

        # ------- S via Gram over the raw At slices (independent of mul) ----
        # S[e, f] = sum_{h, l} At[l, (e, h)] * At[l, (f, h)]
        sg_ps = psum.tile([E, E], f32, tag="sg", bufs=1, name="sg_ps")
        n_acc = LT * NH
        k = 0
        for lt in range(LT):
            at3h = at_sb[:, lt, :].rearrange("p (e h) -> p h e", h=NH)
            for h in range(NH):
                sl = at3h[:, h, :]
                nc.tensor.matmul(sg_ps, sl, sl, start=(k == 0),
                                 stop=(k == n_acc - 1))
                k += 1
        r2_sb = const.tile([E, E], f32, tag="r2_sb")
        nc.scalar.copy(r2_sb, sg_ps)
        nc.vector.reciprocal(r2_sb, r2_sb)
        # flatten [32, 32] -> [1, 1024] via a DRAM bounce, then broadcast to
        # all 128 partitions with ones[1,128].T @ chunk.
        nc.sync.dma_start(out=recd, in_=r2_sb)
        rec1 = const.tile([1, EF], f32, tag="rec1")
        nc.sync.dma_start(out=rec1,
                          in_=recd.rearrange("a b -> (a b)")[None, :])
        recS_sb = big.tile([128, EF], bf16, tag="recS_sb")
        for c in range(2):
            rb = psum.tile([128, HC], f32, tag="sg", bufs=1, name="recB")
            nc.tensor.matmul(rb, ones_row, rec1[:, c * HC:(c + 1) * HC],
                             start=True, stop=True)
            nc.scalar.copy(recS_sb[:, c * HC:(c + 1) * HC], rb)

        # ------- entity projections, transposed: epHT[e', h''] ----------
        # epHT[e', w, h''] = sum_h' entT[h', e'] * W_w[h', h'']
        epHT_sb = const.tile([E, 2, H], bf16, tag="epHT_sb")
        for w, wsb in ((0, wh_sb), (1, wt_sb)):
            for nh in range(2):   # two 384-wide halves of h''
                ps = psum.tile([E, H // 2], f32, tag="tail", bufs=1,
                               name="ep_ps")
                for kt in range(HT):
                    nc.tensor.matmul(
                        ps, entT_sb[:, kt, :],
                        wsb[:, kt, nh * (H // 2):(nh + 1) * (H // 2)],
                        start=(kt == 0), stop=(kt == HT - 1))
                nc.scalar.copy(
                    epHT_sb[:, w, nh * (H // 2):(nh + 1) * (H // 2)], ps)

        # ---------------- chunked main pipeline ----------------
        # Chunk c covers pairs ef in [c*512, (c+1)*512) i.e. e in [16c, 16c+16).

        def emit_mul_chunk(c, lt, mulA=None):
            """Pairwise products + h-tree-sum for chunk c, l-tile lt.

            P[l, e, f, h] = At[l,e,h]*At[l,f,h] with h innermost so both
            operands are stride-1 (DVE 2x mode in bf16). Tree: 12 -> 4
            (two adds) -> 2 -> 1; the last 1x add runs on GpSimd.

            Chunk 1 exploits symmetry: its f<16 half equals the transpose of
            chunk 0's f>=16 half, so only the (e>=16, f>=16) quadrant is
            computed; the rest is one strided copy from the chunk-0 tile.
            """
            at3 = at_sb[:, lt, :].rearrange("p (e h) -> p e h", h=NH)
            mt = mulp.tile([128, HC], bf16, tag="mul", name=f"mul{c}_{lt}")
            m3 = mt.rearrange("p (e f) -> p e f", e=EC)
            es = c * EC
            fs = 0 if c == 0 else EC
            FW = E - fs
            a_e = at3[:, es:es + EC, None, :].broadcast_to([128, EC, FW, NH])
            a_f = at3[:, None, fs:, :].broadcast_to([128, EC, FW, NH])
            pt = prodp.tile([128, EC, FW, NH], bf16, tag=f"prod{c}",
                            name=f"prod{c}_{lt}")
            nc.vector.tensor_mul(pt, a_e, a_f)
            g4 = treep.tile([128, EC, FW, 4], bf16, tag=f"g4_{c}",
                            name=f"g4_{c}_{lt}")
            nc.vector.tensor_add(g4, pt[:, :, :, 0:4], pt[:, :, :, 4:8])
            nc.vector.tensor_add(g4, g4, pt[:, :, :, 8:12])
            h2 = treep.tile([128, EC, FW, 2], bf16, tag=f"h2_{c}",
                            name=f"h2_{c}_{lt}")
            nc.vector.tensor_add(h2, g4[:, :, :, 0:2], g4[:, :, :, 2:4])
            nc.gpsimd.tensor_add(m3[:, :, fs:], h2[:, :, :, 0], h2[:, :, :, 1])
            if c == 1:
                # m3[e2, f1] = mulA[f1, 16+e2] for f1 < 16 (Gram symmetry)
                w = mulA.rearrange("p (e f) -> p e f", e=EC)[:, :, EC:]
                nc.gpsimd.tensor_copy(m3[:, :, :EC],
                                      w.rearrange("p a b -> p b a"))
            return mt

        def emit_ctx_chunk(c, lt, mt, ctx_ps):
            for ht in range(HT):
                nc.tensor.matmul(
                    ctx_ps[ht], seq_sb[:, lt, ht * 128:(ht + 1) * 128],
                    mt, start=(lt == 0), stop=(lt == LT - 1))

        def emit_norm_chunk(c, ctx_ps):
            # ScalarE moves PSUM->SBUF (casting to bf16), DVE scales by 1/S
            # in-place at the bf16 2x rate.
            cn = ctxp.tile([128, HT, HC], bf16, tag="ctxn", name=f"ctxn{c}")
            for ht in range(HT):
                nc.scalar.copy(cn[:, ht, :], ctx_ps[ht])
                nc.vector.tensor_mul(cn[:, ht, :], cn[:, ht, :],
                                     recS_sb[:, c * HC:(c + 1) * HC])
            return cn

        def emit_proj_group(c, g, cn, cand_t, ps_tag):
            w, ht2 = divmod(g, HT)
            wsb = wh_sb if w == 0 else wt_sb
            nb = HT if ps_tag == "ctx" else 1
            ps = psum.tile([128, HC], f32, tag=ps_tag, bufs=nb,
                           name=f"proj{c}_{g}")
            # bias: ep[e or f, h''] broadcast over pair columns via the
            # 0/1 selection matrix (PE accumulate, K=32)
            sel_sl = sel_sb[:, w * EF + c * HC: w * EF + (c + 1) * HC]
            nc.tensor.matmul(ps, epHT_sb[:, w, ht2 * 128:(ht2 + 1) * 128],
                             sel_sl, start=True, stop=False)
            for kt in range(HT):
                nc.tensor.matmul(ps, wsb[:, HT + kt, ht2 * 128:(ht2 + 1) * 128],
                                 cn[:, kt, :],
                                 start=False, stop=(kt == HT - 1))
            cd = candp.tile([128, HC], bf16, tag="cand", name=f"cand{c}_{g}")
            cand_t[g] = cd
            nc.scalar.activation(cd, ps, Act.Tanh)

        def emit_scores_chunk(c, cand_t, ps_tag):
            sc = psum.tile([RP, HC], f32, tag=ps_tag, bufs=1, name=f"sc{c}")
            order = [w * HT + kt for w in range(2) for kt in range(HT)]
            for i, g in enumerate(order):
                nc.tensor.matmul(sc, ptT_sb[:, g, :], cand_t[g],
                                 start=(i == 0), stop=(i == 2 * HT - 1))
            scT = const.tile([RP, HC], f32, tag=f"scT{c}", name=f"scT{c}")
            nc.scalar.copy(scT, sc)
            ob = const.tile([128, LT // 2, R], f32, tag=f"ob{c}",
                            name=f"ob{c}")
            for et in range(LT // 2):
                tp = psum.tile([128, RP], f32, tag="sg", bufs=1, name="tp")
                nc.tensor.transpose(tp, scT[:, et * 128:(et + 1) * 128],
                                    ident)
                nc.vector.tensor_reduce(
                    out=ob[:, et, :],
                    in_=tp.rearrange("p (r q) -> p r q", r=R),
                    axis=Ax.X, op=Alu.max)
            nc.sync.dma_start(
                out=out_d.rearrange("(t p) r -> p t r", p=128)[
                    :, c * (LT // 2):(c + 1) * (LT // 2), :],
                in_=ob)

        # ---- phase A: mul+ctx for chunk 0 ----
        ctxA_ps = [psum.tile([128, HC], f32, tag="ctx", bufs=HT,
                             name=f"ctxA{ht}") for ht in range(HT)]
        mulA_t = []
        for lt in range(LT):
            mt = emit_mul_chunk(0, lt)
            mulA_t.append(mt)
            emit_ctx_chunk(0, lt, mt, ctxA_ps)
        cnA = emit_norm_chunk(0, ctxA_ps)

        # ---- phase B: mul+ctx for chunk 1, interleaved with chunk-0 tail ---
        candA = [None] * (2 * HT)
        ctxB_ps = [psum.tile([128, HC], f32, tag="ctx", bufs=HT,
                             name=f"ctxB{ht}") for ht in range(HT)]
        projA_sched = {1: [0, 1], 2: [2, 3], 3: [4, 5], 4: [6, 7],
                       5: [8, 9], 6: [10, 11]}
        for lt in range(LT):
            mt = emit_mul_chunk(1, lt, mulA=mulA_t[lt])
            emit_ctx_chunk(1, lt, mt, ctxB_ps)
            for g in projA_sched.get(lt, []):
                emit_proj_group(0, g, cnA, candA, "sg" if g % 2 == 0
                                else "tail")
        emit_scores_chunk(0, candA, "tail")
        cnB = emit_norm_chunk(1, ctxB_ps)

        # ---- chunk-1 tail (PE slots from the freed ctx accumulators) ----
        candB = [None] * (2 * HT)
        for g in range(2 * HT):
            emit_proj_group(1, g, cnB, candB, "ctx")
        emit_scores_chunk(1, candB, "tail")


def _host_prep(sequence_output, attention, W_head, W_tail, prototypes,
               mention_pos):
    """Build the per-core input maps (numpy only)."""
    import ml_dtypes
    bf16 = ml_dtypes.bfloat16

    seq = np.asarray(sequence_output, dtype=np.float32)
    att = np.asarray(attention, dtype=np.float32)
    wh = np.ascontiguousarray(W_head, dtype=np.float32).astype(bf16)
    wt = np.ascontiguousarray(W_tail, dtype=np.float32).astype(bf16)
    pro = np.asarray(prototypes, dtype=np.float32)
    pos = np.asarray(mention_pos)

    # selection matrices: selH[e', (e,f)] = [e'==e], selT[f', (e,f)] = [f'==f]
    eye = np.eye(E, dtype=np.float32)
    selH = np.repeat(eye, E, axis=1)           # [E, EF], col ef -> e
    selT = np.tile(eye, (1, E))                # [E, EF], col ef -> f
    sel = np.ascontiguousarray(
        np.concatenate([selH, selT], axis=1)).astype(bf16)

    in_maps = []
    for c in range(NCORES):
        b, q = divmod(c, Q)
        p_bq = pos[b, q]                       # [E, M]
        # attention gather + mention-sum: [NH, E, L] (scale dropped)
        g = att[b, q][:, p_bq, :]              # [NH, E, M, L]
        asum = g[:, :, 0, :] + g[:, :, 1, :]   # [NH, E, L]
        # At[l, e*NH + h] with h innermost (DVE 2x-mode layout)
        at = np.ascontiguousarray(
            asum.transpose(2, 1, 0).reshape(L, E * NH)).astype(bf16)
        # entity means: [E, H] -> entT [H, E]
        ment = seq[b, q][p_bq]                 # [E, M, H]
        ent = (ment[:, 0, :] + ment[:, 1, :]) * np.float32(0.5)
        entT = np.ascontiguousarray(ent.T).astype(bf16)
        ptT = np.ascontiguousarray(
            pro[b].reshape(RP, 2 * H).T).astype(bf16)   # [2H, RP]
        in_maps.append({
            "at": at,
            "seq": seq[b, q].astype(bf16),
            "entT": entT,
            "wh": wh,
            "wt": wt,
            "ptT": ptT,
            "sel": sel,
        })
    return in_maps


def kernel(sequence_output, attention, W_head, W_tail, prototypes,
           mention_pos):
    from concourse.bass_utils import run_bass_kernel_spmd

    if "nc" not in _CACHE:
        _CACHE["nc"] = _build_program()
    nc = _CACHE["nc"]

    in_maps = _host_prep(sequence_output, attention, W_head, W_tail,
                         prototypes, mention_pos)
    res = run_bass_kernel_spmd(nc, in_maps, core_ids=list(range(NCORES)))

    out = np.empty((B, Q, E, E, R), dtype=np.float32)
    for c in range(NCORES):
        b, q = divmod(c, Q)
        out[c // Q, c % Q] = res.results[c]["out"].reshape(E, E, R)
    return out


# revision 5
# speedup vs baseline: 1.0686x; 1.0686x over previous
"""Trainium2 Bass kernel for nn_BaseEncoder (ragged entity-pair encoder).

Contract: kernel(**inputs) takes the FULL unsharded inputs (numpy) and
returns the FULL output [B, Q, E, E, R] float32.

Sharding: B*Q = 8 independent (batch, query) pairs -> one per NeuronCore.
Small weights (W_head / W_tail / prototypes-for-that-b) are replicated.

Host-side prep per core (cheap, index/layout only):
  - gather the E*M mention rows of the per-query attention and sum over the
    M=2 mentions (the /2 and /NH scalings cancel in the later row-softmax-
    style normalization, so they are dropped),
  - layout At[l, e*NH + h] (h innermost) in bf16 so the device pairwise
    product runs in the DVE 2x perf mode with stride-1 operands,
  - entity means ent = mean_m seq[pos] (transposed to entT),
  - prototypes for this b, reshaped/transposed to [2H, RP],
  - constant 0/1 selection matrices selH/selT that broadcast the entity
    projections over pair columns via a PE matmul (bias-add on PE).

Device kernel per core, pipelined over 4 e-chunks of 8 so each chunk's
norm/proj/tanh/scores tail overlaps the next chunk's DVE mul phase:
  P[l, e, f, h] = At[l,e,h] * At[l,f,h]                       (VectorE, bf16 2x)
  mul[l, ef]    = tree-sum_h P   (symmetric quads copied)     (VectorE + GpSimd)
  S[ef]   = sum_{l,h} At*At   (Gram)                          (TensorE)
  ctxT[h', ef] = sum_l seq[l, h'] * mul[l, ef]                (TensorE)
  ctxnT = ctxT * (1/S)                                        (ScalarE+VectorE)
  epHT[e', h''] = sum_h' entT[h', e'] W[h', h'']              (TensorE)
  proj psum = selH/T-bias MM + sum_kt W2^T ctxnT              (TensorE)
  cand = tanh(psum)                                           (ScalarE)
  scores[ef, rp] = sum_d candT[d, ef] * protoT[d, rp]         (TensorE)
  out[ef, r] = max_p scores[ef, r*10+p]                       (VectorE)
"""

import numpy as np

B, Q, L, H, E, M, R, P, NH = 2, 4, 1024, 768, 32, 2, 5, 10, 12
NCORES = 8
LT = L // 128          # 8 l-tiles
HT = H // 128          # 6 tiles of 128 along a hidden dim
EF = E * E             # 1024 entity pairs
RP = R * P             # 50 prototype rows
NCH = 4                # e-chunks
EW = E // NCH          # 8 e-values per chunk
WC = EW * E            # 256 pair columns per chunk

_CACHE = {}


def _build_program():
    import concourse.mybir as mybir
    import concourse.tile as tile
    from concourse import bacc

    f32 = mybir.dt.float32
    bf16 = mybir.dt.bfloat16
    nc = bacc.Bacc("TRN2", target_bir_lowering=False, debug=False,
                   num_devices=NCORES)

    at_d = nc.dram_tensor("at", [L, E * NH], bf16, kind="ExternalInput").ap()
    seq_d = nc.dram_tensor("seq", [L, H], bf16, kind="ExternalInput").ap()
    entT_d = nc.dram_tensor("entT", [H, E], bf16, kind="ExternalInput").ap()
    wh_d = nc.dram_tensor("wh", [2 * H, H], bf16, kind="ExternalInput").ap()
    wt_d = nc.dram_tensor("wt", [2 * H, H], bf16, kind="ExternalInput").ap()
    ptT_d = nc.dram_tensor("ptT", [2 * H, RP], bf16, kind="ExternalInput").ap()
    sel_d = nc.dram_tensor("sel", [E, 2 * EF], bf16, kind="ExternalInput").ap()
    out_d = nc.dram_tensor("out", [EF, R], f32, kind="ExternalOutput").ap()

    with tile.TileContext(nc) as tc:
        _emit(tc, mybir, at_d, seq_d, entT_d, wh_d, wt_d, ptT_d, sel_d, out_d)

    nc.compile()
    return nc


def _emit(tc, mybir, at_d, seq_d, entT_d, wh_d, wt_d, ptT_d, sel_d, out_d):
    nc = tc.nc
    f32 = mybir.dt.float32
    bf16 = mybir.dt.bfloat16

    Alu = mybir.AluOpType
    Act = mybir.ActivationFunctionType
    Ax = mybir.AxisListType
    from concourse.masks import make_identity

    import contextlib
    ctx = contextlib.ExitStack()
    with ctx:
        const = ctx.enter_context(tc.tile_pool(name="const", bufs=1))
        big = ctx.enter_context(tc.tile_pool(name="big", bufs=1))
        mulp = ctx.enter_context(tc.tile_pool(name="mulp", bufs=NCH * LT))
        prodp = ctx.enter_context(tc.tile_pool(name="prodp", bufs=2))
        treep = ctx.enter_context(tc.tile_pool(name="treep", bufs=2))
        candp = ctx.enter_context(tc.tile_pool(name="candp", bufs=16))
        ctxp = ctx.enter_context(tc.tile_pool(name="ctxp", bufs=2))
        # PSUM: 8 banks statically split into tags
        #   "ctx": 6 x 1 bank   (per-chunk ctx accumulators)
        #   "sg":  1 x 1 bank   (S-gram, recS broadcast, even proj groups, tp)
        #   "tail": 1 x 1 bank  (ep, odd proj groups, scores)
        psum = ctx.enter_context(tc.tile_pool(name="psum", bufs=1, space="PSUM"))

        # ---------------- input loads ----------------
        at_sb = big.tile([128, LT, E * NH], bf16, tag="at_sb")
        at_r = at_d.rearrange("(t p) n -> p t n", p=128)
        for lt in range(LT):
            nc.sync.dma_start(out=at_sb[:, lt, :], in_=at_r[:, lt, :])
        seq_sb = big.tile([128, LT, H], bf16, tag="seq_sb")
        nc.sync.dma_start(out=seq_sb, in_=seq_d.rearrange("(t p) n -> p t n", p=128))
        entT_sb = const.tile([128, HT, E], bf16, tag="entT_sb")
        nc.sync.dma_start(out=entT_sb, in_=entT_d.rearrange("(t p) n -> p t n", p=128))
        ptT_sb = const.tile([128, 2 * HT, RP], bf16, tag="ptT_sb")
        nc.sync.dma_start(out=ptT_sb, in_=ptT_d.rearrange("(t p) n -> p t n", p=128))
        wh_sb = big.tile([128, 2 * HT, H], bf16, tag="wh_sb")
        nc.sync.dma_start(out=wh_sb, in_=wh_d.rearrange("(t p) n -> p t n", p=128))
        wt_sb = big.tile([128, 2 * HT, H], bf16, tag="wt_sb")
        nc.sync.dma_start(out=wt_sb, in_=wt_d.rearrange("(t p) n -> p t n", p=128))
        sel_sb = const.tile([E, 2 * EF], bf16, tag="sel_sb")
        nc.sync.dma_start(out=sel_sb, in_=sel_d)

        ones_row = const.tile([1, 128], f32, tag="ones_row")
        nc.vector.memset(ones_row, 1.0)
        ident = const.tile([RP, RP], f32, tag="ident")
        make_identity(nc, ident)
        recd = nc.dram_tensor("recd", [E, E], f32).ap()

        # ------- S via Gram over the raw At slices (independent of mul) ----
        # S[e, f] = sum_{h, l} At[l, (e, h)] * At[l, (f, h)]
        sg_ps = psum.tile([E, E], f32, tag="sg", bufs=1, name="sg_ps")
        n_acc = LT * NH
        k = 0
        for lt in range(LT):
            at3h = at_sb[:, lt, :].rearrange("p (e h) -> p h e", h=NH)
            for h in range(NH):
                sl = at3h[:, h, :]
                nc.tensor.matmul(sg_ps, sl, sl, start=(k == 0),
                                 stop=(k == n_acc - 1))
                k += 1
        r2_sb = const.tile([E, E], f32, tag="r2_sb")
        nc.scalar.copy(r2_sb, sg_ps)
        nc.vector.reciprocal(r2_sb, r2_sb)
        # flatten [32, 32] -> [1, 1024] via a DRAM bounce, then broadcast to
        # all 128 partitions with ones[1,128].T @ chunk.
        nc.sync.dma_start(out=recd, in_=r2_sb)
        rec1 = const.tile([1, EF], f32, tag="rec1")
        nc.sync.dma_start(out=rec1,
                          in_=recd.rearrange("a b -> (a b)")[None, :])
        recS_sb = big.tile([128, EF], bf16, tag="recS_sb")
        for half in range(2):
            rb = psum.tile([128, EF // 2], f32, tag="sg", bufs=1, name="recB")
            nc.tensor.matmul(rb, ones_row,
                             rec1[:, half * (EF // 2):(half + 1) * (EF // 2)],
                             start=True, stop=True)
            nc.scalar.copy(recS_sb[:, half * (EF // 2):(half + 1) * (EF // 2)],
                           rb)

        # ------- entity projections, transposed: epHT[e', h''] ----------
        # epHT[e', w, h''] = sum_h' entT[h', e'] * W_w[h', h'']
        epHT_sb = const.tile([E, 2, H], bf16, tag="epHT_sb")
        for w, wsb in ((0, wh_sb), (1, wt_sb)):
            for nh in range(2):   # two 384-wide halves of h''
                ps = psum.tile([E, H // 2], f32, tag="tail", bufs=1,
                               name="ep_ps")
                for kt in range(HT):
                    nc.tensor.matmul(
                        ps, entT_sb[:, kt, :],
                        wsb[:, kt, nh * (H // 2):(nh + 1) * (H // 2)],
                        start=(kt == 0), stop=(kt == HT - 1))
                nc.scalar.copy(
                    epHT_sb[:, w, nh * (H // 2):(nh + 1) * (H // 2)], ps)

        # ---------------- chunked main pipeline ----------------
        # Chunk c covers e in [8c, 8c+8) i.e. pair columns [256c, 256(c+1)).
        mul_t = [[None] * LT for _ in range(NCH)]

        def emit_mul_chunk(c, lt):
            """Pairwise products + h-tree-sum for chunk c, l-tile lt.

            P[l, e, f, h] = At[l,e,h]*At[l,f,h] with h innermost so both
            operands are stride-1 (DVE 2x mode in bf16). Tree: 12 -> 4
            (two aligned adds) -> 2 -> 1.

            Symmetry: only f >= 8c is computed; the f < 8c quadrants are
            strided transpose-copies out of the earlier chunks' tiles.
            """
            at3 = at_sb[:, lt, :].rearrange("p (e h) -> p e h", h=NH)
            mt = mulp.tile([128, WC], bf16, tag="mul", name=f"mul{c}_{lt}")
            mul_t[c][lt] = mt
            m3 = mt.rearrange("p (e f) -> p e f", e=EW)
            es = c * EW
            fs = c * EW
            FW = E - fs
            # copies first: sources are earlier chunks' tiles (long since
            # written), so these run ahead of this tile's tree ops.
            for cp in range(c):
                src3 = mul_t[cp][lt].rearrange("p (e f) -> p e f", e=EW)
                blk = src3[:, :, es:es + EW]
                nc.vector.tensor_copy(m3[:, :, cp * EW:(cp + 1) * EW],
                                      blk.rearrange("p a b -> p b a"))
            a_e = at3[:, es:es + EW, None, :].broadcast_to([128, EW, FW, NH])
            a_f = at3[:, None, fs:, :].broadcast_to([128, EW, FW, NH])
            pt = prodp.tile([128, EW, FW, NH], bf16, tag=f"prod{c}",
                            name=f"prod{c}_{lt}")
            nc.vector.tensor_mul(pt, a_e, a_f)
            g4 = treep.tile([128, EW, FW, 4], bf16, tag=f"g4_{c}",
                            name=f"g4_{c}_{lt}")
            nc.vector.tensor_add(g4, pt[:, :, :, 0:4], pt[:, :, :, 4:8])
            nc.vector.tensor_add(g4, g4, pt[:, :, :, 8:12])
            h2 = treep.tile([128, EW, FW, 2], bf16, tag=f"h2_{c}",
                            name=f"h2_{c}_{lt}")
            nc.vector.tensor_add(h2, g4[:, :, :, 0:2], g4[:, :, :, 2:4])
            eng = nc.gpsimd if c < 2 else nc.vector
            eng.tensor_add(m3[:, :, fs:], h2[:, :, :, 0], h2[:, :, :, 1])
            return mt

        def emit_ctx_chunk(c, lt, mt, ctx_ps):
            for ht in range(HT):
                nc.tensor.matmul(
                    ctx_ps[ht], seq_sb[:, lt, ht * 128:(ht + 1) * 128],
                    mt, start=(lt == 0), stop=(lt == LT - 1))

        def emit_norm_chunk(c, ctx_ps):
            # ScalarE moves PSUM->SBUF (casting to bf16), DVE scales by 1/S
            # in-place at the bf16 2x rate.
            cn = ctxp.tile([128, HT, WC], bf16, tag="ctxn", name=f"ctxn{c}")
            for ht in range(HT):
                nc.scalar.copy(cn[:, ht, :], ctx_ps[ht])
                nc.vector.tensor_mul(cn[:, ht, :], cn[:, ht, :],
                                     recS_sb[:, c * WC:(c + 1) * WC])
            return cn

        def emit_proj_group(c, g, cn, cand_t, ps_tag):
            w, ht2 = divmod(g, HT)
            wsb = wh_sb if w == 0 else wt_sb
            ps = psum.tile([128, WC], f32, tag=ps_tag, bufs=1,
                           name=f"proj{c}_{g}")
            # bias: ep[e or f, h''] broadcast over pair columns via the
            # 0/1 selection matrix (PE accumulate, K=32)
            sel_sl = sel_sb[:, w * EF + c * WC: w * EF + (c + 1) * WC]
            nc.tensor.matmul(ps, epHT_sb[:, w, ht2 * 128:(ht2 + 1) * 128],
                             sel_sl, start=True, stop=False)
            for kt in range(HT):
                nc.tensor.matmul(ps, wsb[:, HT + kt, ht2 * 128:(ht2 + 1) * 128],
                                 cn[:, kt, :],
                                 start=False, stop=(kt == HT - 1))
            cd = candp.tile([128, WC], bf16, tag="cand", name=f"cand{c}_{g}")
            cand_t[g] = cd
            nc.scalar.activation(cd, ps, Act.Tanh)

        def emit_scores_chunk(c, cand_t, ps_tag):
            sc = psum.tile([RP, WC], f32, tag=ps_tag, bufs=1, name=f"sc{c}")
            order = [w * HT + kt for w in range(2) for kt in range(HT)]
            for i, g in enumerate(order):
                nc.tensor.matmul(sc, ptT_sb[:, g, :], cand_t[g],
                                 start=(i == 0), stop=(i == 2 * HT - 1))
            scT = const.tile([RP, WC], f32, tag=f"scT{c}", name=f"scT{c}")
            nc.scalar.copy(scT, sc)
            nt = WC // 128
            ob = const.tile([128, nt, R], f32, tag=f"ob{c}", name=f"ob{c}")
            for et in range(nt):
                tp = psum.tile([128, RP], f32, tag="sg", bufs=1, name="tp")
                nc.tensor.transpose(tp, scT[:, et * 128:(et + 1) * 128],
                                    ident)
                nc.vector.tensor_reduce(
                    out=ob[:, et, :],
                    in_=tp.rearrange("p (r q) -> p r q", r=R),
                    axis=Ax.X, op=Alu.max)
            nc.sync.dma_start(
                out=out_d.rearrange("(t p) r -> p t r", p=128)[
                    :, c * nt:(c + 1) * nt, :],
                in_=ob)

        # ---- pipelined phases: chunk c's mul+ctx overlap chunk c-1's tail --
        proj_sched = {1: [0, 1], 2: [2, 3], 3: [4, 5], 4: [6, 7],
                      5: [8, 9], 6: [10, 11]}
        cn_t = [None] * NCH
        cand_t = [[None] * (2 * HT) for _ in range(NCH)]
        for c in range(NCH):
            ctx_ps = [psum.tile([128, WC], f32, tag="ctx", bufs=HT,
                                name=f"ctx{c}_{ht}") for ht in range(HT)]
            for lt in range(LT):
                mt = emit_mul_chunk(c, lt)
                emit_ctx_chunk(c, lt, mt, ctx_ps)
                if c >= 1:
                    for g in proj_sched.get(lt, []):
                        emit_proj_group(c - 1, g, cn_t[c - 1], cand_t[c - 1],
                                        "sg" if g % 2 == 0 else "tail")
            if c >= 1:
                emit_scores_chunk(c - 1, cand_t[c - 1], "tail")
            cn_t[c] = emit_norm_chunk(c, ctx_ps)

        # ---- last chunk's tail ----
        for g in range(2 * HT):
            emit_proj_group(NCH - 1, g, cn_t[NCH - 1], cand_t[NCH - 1],
                            "sg" if g % 2 == 0 else "tail")
        emit_scores_chunk(NCH - 1, cand_t[NCH - 1], "tail")


def _host_prep(sequence_output, attention, W_head, W_tail, prototypes,
               mention_pos):
    """Build the per-core input maps (numpy only)."""
    import ml_dtypes
    bf16 = ml_dtypes.bfloat16

    seq = np.asarray(sequence_output, dtype=np.float32)
    att = np.asarray(attention, dtype=np.float32)
    wh = np.ascontiguousarray(W_head, dtype=np.float32).astype(bf16)
    wt = np.ascontiguousarray(W_tail, dtype=np.float32).astype(bf16)
    pro = np.asarray(prototypes, dtype=np.float32)
    pos = np.asarray(mention_pos)

    # selection matrices: selH[e', (e,f)] = [e'==e], selT[f', (e,f)] = [f'==f]
    eye = np.eye(E, dtype=np.float32)
    selH = np.repeat(eye, E, axis=1)           # [E, EF], col ef -> e
    selT = np.tile(eye, (1, E))                # [E, EF], col ef -> f
    sel = np.ascontiguousarray(
        np.concatenate([selH, selT], axis=1)).astype(bf16)

    in_maps = []
    for c in range(NCORES):
        b, q = divmod(c, Q)
        p_bq = pos[b, q]                       # [E, M]
        # attention gather + mention-sum: [NH, E, L] (scale dropped)
        g = att[b, q][:, p_bq, :]              # [NH, E, M, L]
        asum = g[:, :, 0, :] + g[:, :, 1, :]   # [NH, E, L]
        # At[l, e*NH + h] with h innermost (DVE 2x-mode layout)
        at = np.ascontiguousarray(
            asum.transpose(2, 1, 0).reshape(L, E * NH)).astype(bf16)
        # entity means: [E, H] -> entT [H, E]
        ment = seq[b, q][p_bq]                 # [E, M, H]
        ent = (ment[:, 0, :] + ment[:, 1, :]) * np.float32(0.5)
        entT = np.ascontiguousarray(ent.T).astype(bf16)
        ptT = np.ascontiguousarray(
            pro[b].reshape(RP, 2 * H).T).astype(bf16)   # [2H, RP]
        in_maps.append({
            "at": at,
            "seq": seq[b, q].astype(bf16),
            "entT": entT,
            "wh": wh,
            "wt": wt,
            "ptT": ptT,
            "sel": sel,
        })
    return in_maps


def kernel(sequence_output, attention, W_head, W_tail, prototypes,
           mention_pos):
    from concourse.bass_utils import run_bass_kernel_spmd

    if "nc" not in _CACHE:
        _CACHE["nc"] = _build_program()
    nc = _CACHE["nc"]

    in_maps = _host_prep(sequence_output, attention, W_head, W_tail,
                         prototypes, mention_pos)
    res = run_bass_kernel_spmd(nc, in_maps, core_ids=list(range(NCORES)))

    out = np.empty((B, Q, E, E, R), dtype=np.float32)
    for c in range(NCORES):
        b, q = divmod(c, Q)
        out[b, q] = res.results[c]["out"].reshape(E, E, R)
    return out


# revision 11
# speedup vs baseline: 1.1312x; 1.0586x over previous
"""Trainium2 Bass kernel for nn_BaseEncoder (ragged entity-pair encoder).

Contract: kernel(**inputs) takes the FULL unsharded inputs (numpy) and
returns the FULL output [B, Q, E, E, R] float32.

Sharding: B*Q = 8 independent (batch, query) pairs -> one per NeuronCore.
Small weights (W_head / W_tail / prototypes-for-that-b) are replicated.

Host-side prep per core (cheap, index/layout only):
  - gather the E*M mention rows of the per-query attention and sum over the
    M=2 mentions (the /2 and /NH scalings cancel in the later row-softmax-
    style normalization, so they are dropped),
  - layout At[l, e*NH + h] (h innermost) in bf16 so the device pairwise
    product runs in the DVE 2x perf mode with stride-1 operands,
  - entity means ent = mean_m seq[pos] (transposed to entT),
  - prototypes for this b, reshaped/transposed to [2H, RP],
  - constant 0/1 selection matrices selH/selT that broadcast the entity
    projections over pair columns via a PE matmul (bias-add on PE).

Device kernel per core, pipelined over 4 e-chunks of 8 so each chunk's
norm/proj/tanh/scores tail overlaps the next chunk's DVE mul phase:
  P[l, e, f, h] = At[l,e,h] * At[l,f,h]                       (VectorE, bf16 2x)
  mul[l, ef]    = tree-sum_h P   (symmetric quads copied)     (VectorE + GpSimd)
  S[ef]   = sum_{l,h} At*At   (Gram)                          (TensorE)
  ctxT[h', ef] = sum_l seq[l, h'] * mul[l, ef]                (TensorE)
  ctxnT = ctxT * (1/S)                                        (ScalarE+VectorE)
  epHT[e', h''] = sum_h' entT[h', e'] W[h', h'']              (TensorE)
  proj psum = selH/T-bias MM + sum_kt W2^T ctxnT              (TensorE)
  cand = tanh(psum)                                           (ScalarE)
  scores[ef, rp] = sum_d candT[d, ef] * protoT[d, rp]         (TensorE)
  out[ef, r] = max_p scores[ef, r*10+p]                       (VectorE)
"""

import numpy as np

B, Q, L, H, E, M, R, P, NH = 2, 4, 1024, 768, 32, 2, 5, 10, 12
NCORES = 8
LT = L // 128          # 8 l-tiles
HT = H // 128          # 6 tiles of 128 along a hidden dim
EF = E * E             # 1024 entity pairs
RP = R * P             # 50 prototype rows
NCH = 4                # e-chunks
EW = E // NCH          # 8 e-values per chunk
WC = EW * E            # 256 pair columns per chunk

_CACHE = {}


def _build_program():
    import concourse.mybir as mybir
    import concourse.tile as tile
    from concourse import bacc

    f32 = mybir.dt.float32
    bf16 = mybir.dt.bfloat16
    nc = bacc.Bacc("TRN2", target_bir_lowering=False, debug=False,
                   num_devices=NCORES)

    at_d = nc.dram_tensor("at", [L, E * NH], bf16, kind="ExternalInput").ap()
    seq_d = nc.dram_tensor("seq", [L, H], bf16, kind="ExternalInput").ap()
    entT_d = nc.dram_tensor("entT", [H, E], bf16, kind="ExternalInput").ap()
    wh_d = nc.dram_tensor("wh", [2 * H, H], bf16, kind="ExternalInput").ap()
    wt_d = nc.dram_tensor("wt", [2 * H, H], bf16, kind="ExternalInput").ap()
    ptT_d = nc.dram_tensor("ptT", [2 * H, RP], bf16, kind="ExternalInput").ap()
    sel_d = nc.dram_tensor("sel", [E, 2 * EF], bf16, kind="ExternalInput").ap()
    out_d = nc.dram_tensor("out", [EF, R], f32, kind="ExternalOutput").ap()

    with tile.TileContext(nc) as tc:
        _emit(tc, mybir, at_d, seq_d, entT_d, wh_d, wt_d, ptT_d, sel_d, out_d)

    nc.compile()
    return nc


def _emit(tc, mybir, at_d, seq_d, entT_d, wh_d, wt_d, ptT_d, sel_d, out_d):
    nc = tc.nc
    f32 = mybir.dt.float32
    bf16 = mybir.dt.bfloat16

    Alu = mybir.AluOpType
    Act = mybir.ActivationFunctionType
    Ax = mybir.AxisListType
    from concourse.masks import make_identity

    import contextlib
    ctx = contextlib.ExitStack()
    with ctx:
        const = ctx.enter_context(tc.tile_pool(name="const", bufs=1))
        big = ctx.enter_context(tc.tile_pool(name="big", bufs=1))
        mulp = ctx.enter_context(tc.tile_pool(name="mulp", bufs=NCH * LT))
        prodp = ctx.enter_context(tc.tile_pool(name="prodp", bufs=2))
        treep = ctx.enter_context(tc.tile_pool(name="treep", bufs=2))
        candp = ctx.enter_context(tc.tile_pool(name="candp", bufs=16))
        ctxp = ctx.enter_context(tc.tile_pool(name="ctxp", bufs=2))
        # PSUM: 8 banks statically split into tags
        #   "ctx": 6 x 1 bank   (per-chunk ctx accumulators)
        #   "sg":  1 x 1 bank   (S-gram, recS broadcast, even proj groups, tp)
        #   "tail": 1 x 1 bank  (ep, odd proj groups, scores)
        psum = ctx.enter_context(tc.tile_pool(name="psum", bufs=1, space="PSUM"))

        # ---------------- input loads ----------------
        at_sb = big.tile([128, LT, E * NH], bf16, tag="at_sb")
        at_r = at_d.rearrange("(t p) n -> p t n", p=128)
        for lt in range(LT):
            nc.sync.dma_start(out=at_sb[:, lt, :], in_=at_r[:, lt, :])
        seq_sb = big.tile([128, LT, H], bf16, tag="seq_sb")
        nc.sync.dma_start(out=seq_sb, in_=seq_d.rearrange("(t p) n -> p t n", p=128))
        entT_sb = const.tile([128, HT, E], bf16, tag="entT_sb")
        nc.sync.dma_start(out=entT_sb, in_=entT_d.rearrange("(t p) n -> p t n", p=128))
        ptT_sb = const.tile([128, 2 * HT, RP], bf16, tag="ptT_sb")
        nc.sync.dma_start(out=ptT_sb, in_=ptT_d.rearrange("(t p) n -> p t n", p=128))
        wh_sb = big.tile([128, 2 * HT, H], bf16, tag="wh_sb")
        nc.sync.dma_start(out=wh_sb, in_=wh_d.rearrange("(t p) n -> p t n", p=128))
        wt_sb = big.tile([128, 2 * HT, H], bf16, tag="wt_sb")
        nc.sync.dma_start(out=wt_sb, in_=wt_d.rearrange("(t p) n -> p t n", p=128))
        sel_sb = const.tile([E, 2 * EF], bf16, tag="sel_sb")
        nc.sync.dma_start(out=sel_sb, in_=sel_d)

        ones_row = const.tile([1, 128], f32, tag="ones_row")
        nc.vector.memset(ones_row, 1.0)
        ident = const.tile([RP, RP], f32, tag="ident")
        make_identity(nc, ident)
        recd = nc.dram_tensor("recd", [E, E], f32).ap()

        # ------- S via Gram over the raw At slices (independent of mul) ----
        # S[e, f] = sum_{h, l} At[l, (e, h)] * At[l, (f, h)]
        sg_ps = psum.tile([E, E], f32, tag="sg", bufs=1, name="sg_ps")
        n_acc = LT * NH
        k = 0
        for lt in range(LT):
            at3h = at_sb[:, lt, :].rearrange("p (e h) -> p h e", h=NH)
            for h in range(NH):
                sl = at3h[:, h, :]
                nc.tensor.matmul(sg_ps, sl, sl, start=(k == 0),
                                 stop=(k == n_acc - 1))
                k += 1
        r2_sb = const.tile([E, E], f32, tag="r2_sb")
        nc.scalar.copy(r2_sb, sg_ps)
        nc.vector.reciprocal(r2_sb, r2_sb)
        # flatten [32, 32] -> [1, 1024] via a DRAM bounce, then broadcast to
        # all 128 partitions with ones[1,128].T @ chunk.
        nc.sync.dma_start(out=recd, in_=r2_sb)
        rec1 = const.tile([1, EF], f32, tag="rec1")
        nc.sync.dma_start(out=rec1,
                          in_=recd.rearrange("a b -> (a b)")[None, :])
        recS_sb = big.tile([128, EF], bf16, tag="recS_sb")
        for half in range(2):
            rb = psum.tile([128, EF // 2], f32, tag="sg", bufs=1, name="recB")
            nc.tensor.matmul(rb, ones_row,
                             rec1[:, half * (EF // 2):(half + 1) * (EF // 2)],
                             start=True, stop=True)
            nc.scalar.copy(recS_sb[:, half * (EF // 2):(half + 1) * (EF // 2)],
                           rb)

        # ------- entity projections, transposed: epHT[e', h''] ----------
        # epHT[e', w, h''] = sum_h' entT[h', e'] * W_w[h', h'']
        epHT_sb = const.tile([E, 2, H], bf16, tag="epHT_sb")
        for w, wsb in ((0, wh_sb), (1, wt_sb)):
            for nh in range(2):   # two 384-wide halves of h''
                ps = psum.tile([E, H // 2], f32, tag="tail", bufs=1,
                               name="ep_ps")
                for kt in range(HT):
                    nc.tensor.matmul(
                        ps, entT_sb[:, kt, :],
                        wsb[:, kt, nh * (H // 2):(nh + 1) * (H // 2)],
                        start=(kt == 0), stop=(kt == HT - 1))
                nc.scalar.copy(
                    epHT_sb[:, w, nh * (H // 2):(nh + 1) * (H // 2)], ps)

        # ---------------- chunked main pipeline ----------------
        # Chunk c covers e in [8c, 8c+8) i.e. pair columns [256c, 256(c+1)).
        mul_t = [[None] * LT for _ in range(NCH)]

        def emit_mul_chunk(c, lt):
            """Pairwise products + h-tree-sum for chunk c, l-tile lt.

            P[l, e, f, h] = At[l,e,h]*At[l,f,h] with h innermost so both
            operands are stride-1 (DVE 2x mode in bf16). Tree: 12 -> 4
            (two aligned adds) -> 2 -> 1.

            Symmetry: only f >= 8c is computed; the f < 8c quadrants are
            strided transpose-copies out of the earlier chunks' tiles.
            """
            at3 = at_sb[:, lt, :].rearrange("p (e h) -> p e h", h=NH)
            mt = mulp.tile([128, WC], bf16, tag="mul", name=f"mul{c}_{lt}")
            mul_t[c][lt] = mt
            m3 = mt.rearrange("p (e f) -> p e f", e=EW)
            es = c * EW
            fs = c * EW
            FW = E - fs
            # copies first: sources are earlier chunks' tiles (long since
            # written). ScalarE does them - it has slack and this keeps the
            # in-order DVE queue free for the product/tree stream.
            for cp in range(c):
                src3 = mul_t[cp][lt].rearrange("p (e f) -> p e f", e=EW)
                blk = src3[:, :, es:es + EW]
                nc.scalar.copy(m3[:, :, cp * EW:(cp + 1) * EW],
                               blk.rearrange("p a b -> p b a"))
            a_e = at3[:, es:es + EW, None, :].broadcast_to([128, EW, FW, NH])
            a_f = at3[:, None, fs:, :].broadcast_to([128, EW, FW, NH])
            pt = prodp.tile([128, EW, FW, NH], bf16, tag=f"prod{c}",
                            name=f"prod{c}_{lt}")
            nc.vector.tensor_mul(pt, a_e, a_f)
            g4 = treep.tile([128, EW, FW, 4], bf16, tag=f"g4_{c}",
                            name=f"g4_{c}_{lt}")
            nc.vector.tensor_add(g4, pt[:, :, :, 0:4], pt[:, :, :, 4:8])
            nc.vector.tensor_add(g4, g4, pt[:, :, :, 8:12])
            h2 = treep.tile([128, EW, FW, 2], bf16, tag=f"h2_{c}",
                            name=f"h2_{c}_{lt}")
            nc.vector.tensor_add(h2, g4[:, :, :, 0:2], g4[:, :, :, 2:4])
            eng = nc.gpsimd if c < 2 else nc.vector
            eng.tensor_add(m3[:, :, fs:], h2[:, :, :, 0], h2[:, :, :, 1])
            return mt

        def emit_ctx_chunk(c, lt, mt, ctx_ps):
            for ht in range(HT):
                nc.tensor.matmul(
                    ctx_ps[ht], seq_sb[:, lt, ht * 128:(ht + 1) * 128],
                    mt, start=(lt == 0), stop=(lt == LT - 1))

        def emit_norm_copies(c, ctx_ps):
            # ScalarE moves PSUM->SBUF (casting to bf16) ...
            cn = ctxp.tile([128, HT, WC], bf16, tag="ctxn", name=f"ctxn{c}")
            for ht in range(HT):
                nc.scalar.copy(cn[:, ht, :], ctx_ps[ht])
            return cn

        def emit_norm_mults(c, cn):
            # ... DVE scales by 1/S in-place at the bf16 2x rate. Emitted
            # separately so the DVE queue isn't head-of-line blocked.
            for ht in range(HT):
                nc.vector.tensor_mul(cn[:, ht, :], cn[:, ht, :],
                                     recS_sb[:, c * WC:(c + 1) * WC])

        def emit_proj_group(c, g, cn, cand_t, ps_tag):
            w, ht2 = divmod(g, HT)
            wsb = wh_sb if w == 0 else wt_sb
            ps = psum.tile([128, WC], f32, tag=ps_tag, bufs=1,
                           name=f"proj{c}_{g}")
            # bias: ep[e or f, h''] broadcast over pair columns via the
            # 0/1 selection matrix (PE accumulate, K=32)
            sel_sl = sel_sb[:, w * EF + c * WC: w * EF + (c + 1) * WC]
            nc.tensor.matmul(ps, epHT_sb[:, w, ht2 * 128:(ht2 + 1) * 128],
                             sel_sl, start=True, stop=False)
            for kt in range(HT):
                nc.tensor.matmul(ps, wsb[:, HT + kt, ht2 * 128:(ht2 + 1) * 128],
                                 cn[:, kt, :],
                                 start=False, stop=(kt == HT - 1))
            cd = candp.tile([128, WC], bf16, tag="cand", name=f"cand{c}_{g}")
            cand_t[g] = cd
            nc.scalar.activation(cd, ps, Act.Tanh)

        def emit_scores_pe(c, cand_t, ps_tag):
            sc = psum.tile([RP, WC], f32, tag=ps_tag, bufs=1, name=f"sc{c}")
            order = [w * HT + kt for w in range(2) for kt in range(HT)]
            for i, g in enumerate(order):
                nc.tensor.matmul(sc, ptT_sb[:, g, :], cand_t[g],
                                 start=(i == 0), stop=(i == 2 * HT - 1))
            scT = const.tile([RP, WC], f32, tag=f"scT{c}", name=f"scT{c}")
            nc.scalar.copy(scT, sc)
            nt = WC // 128
            tps = []
            for et in range(nt):
                # alternate psum tags so the 2nd transpose doesn't wait on
                # the (deferred) DVE reduce of the 1st
                tp = psum.tile([128, RP], f32, tag="sg" if et == 0 else "tail",
                               bufs=1, name="tp")
                nc.tensor.transpose(tp, scT[:, et * 128:(et + 1) * 128],
                                    ident)
                tps.append(tp)
            return tps

        def emit_scores_dve(c, tps):
            # deferred DVE max-reduce + output DMA (next phase, so the DVE
            # queue isn't blocked waiting on the PE scores/transpose chain)
            nt = WC // 128
            ob = const.tile([128, nt, R], f32, tag=f"ob{c}", name=f"ob{c}")
            for et, tp in enumerate(tps):
                nc.vector.tensor_reduce(
                    out=ob[:, et, :],
                    in_=tp.rearrange("p (r q) -> p r q", r=R),
                    axis=Ax.X, op=Alu.max)
            nc.sync.dma_start(
                out=out_d.rearrange("(t p) r -> p t r", p=128)[
                    :, c * nt:(c + 1) * nt, :],
                in_=ob)

        # ---- pipelined phases: chunk c's mul+ctx overlap chunk c-1's tail --
        # Tail pieces of chunk c-1 (norm, proj, scores) are interleaved into
        # phase c's lt loop; cross-engine-dependent DVE bits (norm mults,
        # score reduces) come AFTER the phase's first mul ops so the in-order
        # DVE queue never stalls at a phase boundary.
        proj_sched = {2: [0, 1], 3: [2, 3], 4: [4, 5], 5: [6, 7],
                      6: [8, 9], 7: [10, 11]}
        cn_t = [None] * NCH
        cand_t = [[None] * (2 * HT) for _ in range(NCH)]
        tps_t = [None] * NCH
        for c in range(NCH):
            ctx_ps = [psum.tile([128, WC], f32, tag="ctx", bufs=HT,
                                name=f"ctx{c}_{ht}") for ht in range(HT)]
            for lt in range(LT):
                mt = emit_mul_chunk(c, lt)
                emit_ctx_chunk(c, lt, mt, ctx_ps)
                if lt == 0:
                    if c >= 1:
                        cn_t[c - 1] = emit_norm_copies(c - 1, prev_ctx_ps)
                    if c >= 2 and tps_t[c - 2] is not None:
                        emit_scores_dve(c - 2, tps_t[c - 2])
                elif lt == 1 and c >= 1:
                    emit_norm_mults(c - 1, cn_t[c - 1])
                if c >= 1:
                    for g in proj_sched.get(lt, []):
                        emit_proj_group(c - 1, g, cn_t[c - 1], cand_t[c - 1],
                                        "sg" if g % 2 == 0 else "tail")
            if c >= 1:
                tps_t[c - 1] = emit_scores_pe(c - 1, cand_t[c - 1], "tail")
            prev_ctx_ps = ctx_ps

        # ---- last chunk's tail ----
        cn_t[NCH - 1] = emit_norm_copies(NCH - 1, prev_ctx_ps)
        emit_norm_mults(NCH - 1, cn_t[NCH - 1])
        for g in range(2 * HT):
            emit_proj_group(NCH - 1, g, cn_t[NCH - 1], cand_t[NCH - 1],
                            "sg" if g % 2 == 0 else "tail")
        emit_scores_dve(NCH - 2, tps_t[NCH - 2])
        tps_t[NCH - 1] = emit_scores_pe(NCH - 1, cand_t[NCH - 1], "tail")
        emit_scores_dve(NCH - 1, tps_t[NCH - 1])


def _host_prep(sequence_output, attention, W_head, W_tail, prototypes,
               mention_pos):
    """Build the per-core input maps (numpy only)."""
    import ml_dtypes
    bf16 = ml_dtypes.bfloat16

    seq = np.asarray(sequence_output, dtype=np.float32)
    att = np.asarray(attention, dtype=np.float32)
    wh = np.ascontiguousarray(W_head, dtype=np.float32).astype(bf16)
    wt = np.ascontiguousarray(W_tail, dtype=np.float32).astype(bf16)
    pro = np.asarray(prototypes, dtype=np.float32)
    pos = np.asarray(mention_pos)

    # selection matrices: selH[e', (e,f)] = [e'==e], selT[f', (e,f)] = [f'==f]
    eye = np.eye(E, dtype=np.float32)
    selH = np.repeat(eye, E, axis=1)           # [E, EF], col ef -> e
    selT = np.tile(eye, (1, E))                # [E, EF], col ef -> f
    sel = np.ascontiguousarray(
        np.concatenate([selH, selT], axis=1)).astype(bf16)

    in_maps = []
    for c in range(NCORES):
        b, q = divmod(c, Q)
        p_bq = pos[b, q]                       # [E, M]
        # attention gather + mention-sum: [NH, E, L] (scale dropped)
        g = att[b, q][:, p_bq, :]              # [NH, E, M, L]
        asum = g[:, :, 0, :] + g[:, :, 1, :]   # [NH, E, L]
        # At[l, e*NH + h] with h innermost (DVE 2x-mode layout)
        at = np.ascontiguousarray(
            asum.transpose(2, 1, 0).reshape(L, E * NH)).astype(bf16)
        # entity means: [E, H] -> entT [H, E]
        ment = seq[b, q][p_bq]                 # [E, M, H]
        ent = (ment[:, 0, :] + ment[:, 1, :]) * np.float32(0.5)
        entT = np.ascontiguousarray(ent.T).astype(bf16)
        ptT = np.ascontiguousarray(
            pro[b].reshape(RP, 2 * H).T).astype(bf16)   # [2H, RP]
        in_maps.append({
            "at": at,
            "seq": seq[b, q].astype(bf16),
            "entT": entT,
            "wh": wh,
            "wt": wt,
            "ptT": ptT,
            "sel": sel,
        })
    return in_maps


def kernel(sequence_output, attention, W_head, W_tail, prototypes,
           mention_pos):
    from concourse.bass_utils import run_bass_kernel_spmd

    if "nc" not in _CACHE:
        _CACHE["nc"] = _build_program()
    nc = _CACHE["nc"]

    in_maps = _host_prep(sequence_output, attention, W_head, W_tail,
                         prototypes, mention_pos)
    res = run_bass_kernel_spmd(nc, in_maps, core_ids=list(range(NCORES)))

    out = np.empty((B, Q, E, E, R), dtype=np.float32)
    for c in range(NCORES):
        b, q = divmod(c, Q)
        out[b, q] = res.results[c]["out"].reshape(E, E, R)
    return out
